# revision 42
# baseline (speedup 1.0000x reference)
"""Trainium2 Bass kernel: Bahdanau attention, data-parallel over batch on 8 NeuronCores.

kernel(**inputs) takes the full unsharded inputs (as in reference.setup_inputs())
and returns (context_vector [64, 2048] f32, attention_weights [64, 1024, 1] f32).

Sharding: batch 64 -> 8 per core; small weights replicated. Host-side work is
layout-only (slicing + transposes); all math runs on device.

Per-core device program (B_LOC = 8 batch items), all fp32:
  projhT[a,b] = b1[a] + b2[a] + sum_d W2[d,a] hidden[b,d]      (TensorE)
  ahT[a,l]    = tanh(sum_e W1[e,a] featT[e,l] + projhT[a,b])   (TensorE + ScalarE bias)
  score[l]    = sum_a V[a] ahT[a,l]                            (TensorE)
  E[l], S     = exp(score[l] + bv), sum_l E[l]                 (ScalarE accum_out)
  aw[l]       = E[l] * (1/S)                                   (VectorE)
  ctxT[e]     = (sum_l featT[e,l] E[l]) * (1/S)                (VectorE mult + ScalarE accum)

featT arrives pre-transposed [E, L] so the big matmul streams feature columns
with W1 tiles stationary; the context reduction runs on VectorE (elementwise
multiply) + ScalarE (Copy with accum_out row-reduce), overlapping the next
batch's TensorE work.

Hardware quirks honored here:
  - custom DVE microcode ops (tensor_tensor_reduce etc.) hang this target's
    exec units -> only standard DVE/ACT/PE instructions are used.
  - fp32 matmuls lower to LOW+HIGH double passes at half stream rate (4x
    slower than bf16, measured 858ns vs 213ns per [128]x[128,512] MM)
    -> the streaming matmuls (proj/score/Ebcast) run in bf16 with fp32 PSUM
    accumulation; biases, tanh/exp, softmax and outputs stay fp32.
  - an fp32 matmul is a single self-loading instruction with ONE sync-wait
    slot; tiny "absorber" matmuls/copies first-touch freshly DMA'd tiles so
    no real matmul ever needs two semaphore waits.
"""

import os
import numpy as np
import ml_dtypes
from contextlib import ExitStack

import concourse.bass as bass
import concourse.tile as tile
from concourse import bacc, mybir
from concourse.bass_utils import run_bass_kernel_spmd

FP32 = mybir.dt.float32
BF16 = mybir.dt.bfloat16
AF = mybir.ActivationFunctionType
ALU = mybir.AluOpType

N_CORES = 8
B_LOC, L, ENC, DEC, ATT = 8, 1024, 2048, 512, 512
NET = ENC // 128   # 16 e-tiles
NAT = ATT // 128   # 4 a-tiles
NDT = DEC // 128   # 4 d-tiles
NQ = 4             # feature "quarters" (4 e-tiles each) for DMA pipelining

LAST_EXEC_NS = None
LAST_RESULTS = None


def build_nc(debug=False):
    nc = bacc.Bacc(None, target_bir_lowering=False, debug=debug)

    featT = nc.declare_dram_parameter("featT", [B_LOC, ENC, L], BF16, isOutput=False)
    hiddenT = nc.declare_dram_parameter("hiddenT", [DEC, B_LOC], FP32, isOutput=False)
    W1 = nc.declare_dram_parameter("W1", [ENC, ATT], BF16, isOutput=False)
    W2 = nc.declare_dram_parameter("W2", [DEC, ATT], FP32, isOutput=False)
    b1 = nc.declare_dram_parameter("b1", [1, ATT], FP32, isOutput=False)
    b2 = nc.declare_dram_parameter("b2", [1, ATT], FP32, isOutput=False)
    V = nc.declare_dram_parameter("V", [ATT, 1], BF16, isOutput=False)
    bv = nc.declare_dram_parameter("bv", [1, 1], FP32, isOutput=False)
    ctx_out = nc.declare_dram_parameter("ctx_out", [128, B_LOC, NET], FP32, isOutput=True)
    aw_out = nc.declare_dram_parameter("aw_out", [B_LOC, L], FP32, isOutput=True)
    # internal DRAM bounce rows for the E-broadcast (partition-0-step APs are
    # only legal on DRAM); two slots so consecutive batches don't serialize
    ebounce = nc.dram_tensor("ebounce", [2, L], BF16)

    with ExitStack() as ctx:
        tc = ctx.enter_context(tile.TileContext(nc))
        singles = ctx.enter_context(tc.tile_pool(name="singles", bufs=1))
        ps_proj = ctx.enter_context(tc.tile_pool(name="ps_proj", bufs=3, space="PSUM"))
        ps_sc = ctx.enter_context(tc.tile_pool(name="ps_sc", bufs=1, space="PSUM"))

        def pe_absorb(tile_ap):
            """Tiny matmul whose only dependency is `tile_ap`'s producer --
            soaks up that wait on TensorE so the next real matmul needs at
            most one sync wait (fp32 MM hardware limit)."""
            dmy = ps_sc.tile([1, 2, 512], FP32, tag="sc")
            nc.tensor.matmul(
                dmy[0:1, 0, 0:1], tile_ap[0:1, 0:1], tile_ap[0:1, 0:1],
                start=True, stop=True,
            )

        def dve_absorb(tile_ap, junk):
            """Cheap copy that lands `tile_ap`'s DMA wait on VectorE early."""
            nc.vector.tensor_copy(junk[0:1, 0:1], tile_ap[0:1, 0:1])

        # ---------- preload persistent weights / constants ----------
        # streaming-matmul operands arrive as bf16 from the host; W1 is
        # loaded in four a-tile chunks so the first proj matmul only waits
        # for the chunk it needs
        W1sb = singles.tile([128, NAT, NET, 128], BF16)
        W1r = W1.rearrange("(t p) a -> p t a", p=128)
        for at in range(NAT):
            nc.sync.dma_start(
                out=W1sb[:, at, :, :], in_=W1r[:, :, at * 128:(at + 1) * 128]
            )
        Vsb = singles.tile([128, NAT], BF16)
        nc.sync.dma_start(out=Vsb[:], in_=V.rearrange("(t p) o -> p (t o)", p=128))
        bvsb = singles.tile([1, 1], FP32)
        nc.sync.dma_start(out=bvsb[:], in_=bv[:, :])
        ones128f = singles.tile([1, 128], FP32)
        nc.vector.memset(ones128f[:], 1.0)

        projhT = singles.tile([128, NAT, B_LOC], FP32)
        S_all = singles.tile([1, B_LOC], FP32)
        rS_all = singles.tile([1, B_LOC], FP32)
        ctxT = singles.tile([128, B_LOC, NET], FP32)
        junk = singles.tile([1, 1], FP32)

        # soak up the weight-load DMA waits on TensorE before any real matmul
        for at in range(NAT):
            pe_absorb(W1sb[0:1, at, 0, 0:1])
        pe_absorb(Vsb[0:1, 0:1])

        # ---------- projhT[a, b] = b1[a]+b2[a] + sum_d W2[d,a] hidden[b,d] ----------
        W2sb = singles.tile([128, NDT, ATT], FP32)
        nc.sync.dma_start(out=W2sb[:], in_=W2.rearrange("(t p) a -> p t a", p=128))
        hT = singles.tile([128, NDT, B_LOC], FP32)
        nc.sync.dma_start(out=hT[:], in_=hiddenT.rearrange("(t p) b -> p t b", p=128))
        bb_ = singles.tile([1, 2, ATT], FP32)
        nc.sync.dma_start(out=bb_[:, 0, :], in_=b1[:, :])
        nc.sync.dma_start(out=bb_[:, 1, :], in_=b2[:, :])
        ones8 = singles.tile([1, B_LOC], FP32)
        nc.vector.memset(ones8[:], 1.0)
        b12 = singles.tile([1, ATT], FP32)
        nc.vector.tensor_add(b12[:, :], bb_[:, 0, :], bb_[:, 1, :])

        pe_absorb(W2sb[0:1, 0, 0:1])
        pe_absorb(hT[0:1, 0, 0:1])

        for at in range(NAT):
            pph = ps_sc.tile([128, B_LOC], FP32, tag="sc")
            nc.tensor.matmul(
                pph[:, :],
                b12[0:1, at * 128:(at + 1) * 128],
                ones8[:, :],
                start=True, stop=False,
            )
            for dt_ in range(NDT):
                nc.tensor.matmul(
                    pph[:, :],
                    W2sb[:, dt_, at * 128:(at + 1) * 128],
                    hT[:, dt_, :],
                    start=False, stop=(dt_ == NDT - 1),
                )
            nc.scalar.activation(projhT[:, at, :], pph[:, :], AF.Copy)

        featp = ctx.enter_context(tc.tile_pool(name="featp", bufs=2 * NQ))
        ahp = ctx.enter_context(tc.tile_pool(name="ahp", bufs=2))
        ep = ctx.enter_context(tc.tile_pool(name="ep", bufs=1))
        awp = ctx.enter_context(tc.tile_pool(name="awp", bufs=2))
        scrp = ctx.enter_context(tc.tile_pool(name="scrp", bufs=3))
        ebp = ctx.enter_context(tc.tile_pool(name="ebp", bufs=2))

        # ---------- per-batch feature quarter loads ----------
        featq = [[None] * NQ for _ in range(B_LOC)]

        def load_quarter(b, q):
            t_ = featp.tile([128, NET // NQ, L], BF16, tag="fq")
            src = featT[b, q * 512:(q + 1) * 512, :].rearrange(
                "(t p) l -> p t l", p=128
            )
            nc.sync.dma_start(out=t_[:], in_=src)
            return t_

        for q in range(NQ):
            featq[0][q] = load_quarter(0, q)
        for q in range(NQ):
            pe_absorb(featq[0][q][0:1, 0, 0:1])
            dve_absorb(featq[0][q][0:1, 0, 0:1], junk)

        def emit_proj_at(b, ah, at):
            """One a-tile of proj + fused tanh into ahT."""
            pp = ps_proj.tile([128, 2, 512], FP32, tag="pp")
            for et in range(NET):
                q, t_ = et // NQ, et % NQ
                lhsT = W1sb[:, at, et, :]
                for x in range(2):
                    nc.tensor.matmul(
                        pp[:, x, :],
                        lhsT,
                        featq[b][q][:, t_, x * 512:(x + 1) * 512],
                        start=(et == 0), stop=(et == NET - 1),
                    )
            nc.scalar.activation(
                ah[:, at, :, :], pp[:, :, :], AF.Tanh,
                bias=projhT[:, at, b:b + 1],
            )

        def emit_score_a(b, ah):
            """score matmuls -> exp/S -> aw out -> bf16 row copy."""
            ps = ps_sc.tile([1, 2, 512], FP32, tag="sc")
            for at in range(NAT):
                for x in range(2):
                    nc.tensor.matmul(
                        ps[0:1, x, :],
                        Vsb[:, at:at + 1],
                        ah[:, at, x, :],
                        start=(at == 0), stop=(at == NAT - 1),
                    )

            # E = exp(score + bv), S = sum(E)  (softmax without max-subtraction:
            # |score| <= sum|V| + |bv| < 23, exp stays well inside fp32 range)
            E = ep.tile([1, 2, 512], FP32, tag="E")
            nc.scalar.activation(
                E[:, :, :], ps[0:1, :, :], AF.Exp,
                bias=bvsb[0:1, 0:1], accum_out=S_all[0:1, b:b + 1],
            )

            # aw = E / S  (output attention weights)
            nc.vector.reciprocal(rS_all[0:1, b:b + 1], S_all[0:1, b:b + 1])
            aw = awp.tile([1, 2, 512], FP32, tag="aw")
            nc.vector.tensor_scalar_mul(aw[:, :, :], E[:, :, :], rS_all[0:1, b:b + 1])
            nc.sync.dma_start(
                out=aw_out[b:b + 1, :].rearrange("o (x l) -> o x l", x=2),
                in_=aw[:],
            )
            E_bf = ep.tile([1, 2, 512], BF16, tag="Ebf")
            nc.vector.tensor_copy(E_bf[:, :, :], E[:, :, :])
            return E_bf

        def emit_score_b(E_bf):
            """E broadcast to all 128 partitions via SBUF->SBUF DMA with a
            zero-step partition access pattern (DMA engines are idle here;
            keeps TensorE/VectorE out of the broadcast entirely)."""
            from concourse.tile import add_dep_helper
            slot = emit_score_b.flip = getattr(emit_score_b, "flip", 0) ^ 1
            brow = ebounce[slot:slot + 1, :].rearrange("o (x l) -> o x l", x=2)
            d1 = nc.sync.dma_start(out=brow, in_=E_bf[:, :, :])
            eb_sb = ebp.tile([128, L], BF16, tag="ebsb")
            bsrc = bass.AP(
                tensor=brow.tensor, offset=brow.offset,
                ap=[[0, 128], [1, L]],
            )
            d2 = nc.sync.dma_start(out=eb_sb[:], in_=bsrc)
            # Tile does not track DRAM-tensor deps: order read-after-write
            add_dep_helper(d2.ins, d1.ins, True, "ebounce RAW")
            dve_absorb(eb_sb[0:1, 0:1], junk)
            return eb_sb

        def emit_ctx_chunk(b, eb_sb, ets, dve_reduce=True):
            """ctxT[:, b, et] = sum_l featT[e, l] * E[l] for et in ets:
            VectorE multiply into scratch; row-reduce alternates between
            ScalarE (Copy accum_out) and VectorE (tensor_reduce) unless
            dve_reduce is False (tail: keep VectorE free for multiplies)."""
            for et in ets:
                q, t_ = et // NQ, et % NQ
                f = featq[b][q]
                scr = scrp.tile([128, L], BF16, tag="scr")
                nc.vector.tensor_mul(scr[:, :], f[:, t_, :], eb_sb[:, :])
                if dve_reduce and et % 2 == 1:
                    nc.vector.tensor_reduce(
                        ctxT[:, b, et:et + 1], scr[:, :],
                        axis=mybir.AxisListType.X, op=ALU.add,
                    )
                else:
                    nc.scalar.activation(
                        scr[:, :], scr[:, :], AF.Copy,
                        accum_out=ctxT[:, b, et:et + 1],
                    )

        # ---------- main loop: batch b's proj interleaved with batch b-1's
        # score/softmax/context stages so TensorE never waits on the
        # exp->broadcast chain and HAM stays warm.
        # Per-iteration emission order:
        #   score(b-1) | proj(b).at0 | Ebcast(b-1) | proj(b).at1..3 with ctx
        #   chunks of b-1 interleaved | next-batch absorbers ----------
        pending = None  # (b-1, its ah tile)
        for b in range(B_LOC):
            if b + 1 < B_LOC:
                for q in range(NQ):
                    featq[b + 1][q] = load_quarter(b + 1, q)

            if pending is not None:
                pb, pah = pending
                E_bf = emit_score_a(pb, pah)
            ah = ahp.tile([128, NAT, 2, 512], BF16, tag="ah")
            emit_proj_at(b, ah, 0)
            if pending is not None:
                eb_sb = emit_score_b(E_bf)
            emit_proj_at(b, ah, 1)
            if pending is not None:
                emit_ctx_chunk(pb, eb_sb, range(0, 6))
            emit_proj_at(b, ah, 2)
            if pending is not None:
                emit_ctx_chunk(pb, eb_sb, range(6, 12))
            emit_proj_at(b, ah, 3)
            if pending is not None:
                emit_ctx_chunk(pb, eb_sb, range(12, NET))
            # first-touch absorbers for the next batch's feature quarters
            if b + 1 < B_LOC:
                for q in range(NQ):
                    pe_absorb(featq[b + 1][q][0:1, 0, 0:1])
                    dve_absorb(featq[b + 1][q][0:1, 0, 0:1], junk)
            pending = (b, ah)

        pb, pah = pending
        E_bf = emit_score_a(pb, pah)
        eb_sb = emit_score_b(E_bf)
        emit_ctx_chunk(pb, eb_sb, range(NET), dve_reduce=False)

        # ---------- normalize ctx by 1/S and store ----------
        prsb = ps_sc.tile([128, B_LOC], FP32, tag="sc")
        nc.tensor.matmul(prsb[:, :], ones128f[:, :], rS_all[0:1, :], start=True, stop=True)
        rsb = singles.tile([128, B_LOC], FP32)
        nc.scalar.activation(rsb[:, :], prsb[:, :], AF.Copy)
        for b in range(B_LOC):
            nc.vector.tensor_scalar_mul(ctxT[:, b, :], ctxT[:, b, :], rsb[:, b:b + 1])
        nc.sync.dma_start(out=ctx_out[:, :, :], in_=ctxT[:, :, :])

    nc.compile()
    return nc


def shard_inputs(features, hidden_state, W1, b1, W2, b2, V, bv, n_cores=N_CORES):
    """Full inputs -> list of per-core in_maps (host-side layout/precision
    prep only: batch sharding, [L,E]->[E,L] transpose, bf16 cast of the
    streaming-matmul operands)."""
    features = np.ascontiguousarray(features, dtype=np.float32)
    B = features.shape[0]
    per = B // n_cores
    assert per == B_LOC
    bf = ml_dtypes.bfloat16
    w1 = np.ascontiguousarray(np.asarray(W1, np.float32).astype(bf))
    w2 = np.ascontiguousarray(W2, np.float32)
    b1r = np.ascontiguousarray(b1, np.float32).reshape(1, ATT)
    b2r = np.ascontiguousarray(b2, np.float32).reshape(1, ATT)
    vr = np.ascontiguousarray(np.asarray(V, np.float32).astype(bf)).reshape(ATT, 1)
    bvr = np.ascontiguousarray(bv, np.float32).reshape(1, 1)
    in_maps = []
    for c in range(n_cores):
        fs = features[c * per:(c + 1) * per]
        in_maps.append({
            "featT": np.ascontiguousarray(fs.transpose(0, 2, 1).astype(bf)),
            "hiddenT": np.ascontiguousarray(
                np.asarray(hidden_state[c * per:(c + 1) * per], np.float32).T
            ),
            "W1": w1, "W2": w2, "b1": b1r, "b2": b2r, "V": vr, "bv": bvr,
        })
    return in_maps


def gather_outputs(results):
    """Per-core result dicts -> (context_vector [B, ENC], attention_weights [B, L, 1])."""
    ctxs, aws = [], []
    for o in results:
        ctxs.append(
            np.ascontiguousarray(o["ctx_out"]).reshape(128, B_LOC, NET)
            .transpose(1, 2, 0).reshape(B_LOC, ENC)
        )
        aws.append(np.ascontiguousarray(o["aw_out"]).reshape(B_LOC, L, 1))
    return np.concatenate(ctxs, 0), np.concatenate(aws, 0)


_NC_CACHE = {}


def _get_nc():
    if "nc" not in _NC_CACHE:
        _NC_CACHE["nc"] = build_nc(debug=False)
    return _NC_CACHE["nc"]


def _ensure_profile_hook():
    """Provide antenv.axon_hooks + a ctypes NTFF profile hook when the
    environment ships a trimmed antenv (degrades silently when absent)."""
    import sys, types, contextlib, ctypes

    try:
        from antenv.axon_hooks import get_axon_ntff_profile_hook  # noqa: F401
        return
    except ImportError:
        pass

    so_path = "/opt/axon/libaxon_pjrt.so"
    if not os.path.exists(so_path):
        return
    lib = ctypes.CDLL(so_path)
    if not hasattr(lib, "axon_start_nrt_profile"):
        return
    lib.axon_start_nrt_profile.argtypes = [
        ctypes.POINTER(ctypes.c_int64), ctypes.c_size_t,
    ]
    lib.axon_start_nrt_profile.restype = ctypes.c_int64
    lib.axon_stop_nrt_profile.argtypes = [ctypes.c_char_p]
    lib.axon_stop_nrt_profile.restype = ctypes.c_int64

    @contextlib.contextmanager
    def _hook(output_dir, device_ids):
        import jax
        jax.devices()
        if device_ids:
            ids = (ctypes.c_int64 * len(device_ids))(*device_ids)
            rc = lib.axon_start_nrt_profile(ids, len(device_ids))
        else:
            rc = lib.axon_start_nrt_profile(None, 0)
        if rc != 0:
            raise RuntimeError(f"axon_start_nrt_profile rc={rc}")
        try:
            yield
        finally:
            n = lib.axon_stop_nrt_profile(str(output_dir).encode())
            if n < 0:
                raise RuntimeError(f"axon_stop_nrt_profile rc={n}")
            print(f"profile: {n} file(s) written to {output_dir}")

    mod = types.ModuleType("antenv.axon_hooks")
    _state = {"hook": _hook}
    mod.set_axon_ntff_profile_hook = lambda h: _state.__setitem__("hook", h)
    mod.get_axon_ntff_profile_hook = lambda: _state["hook"]
    sys.modules["antenv.axon_hooks"] = mod
    import antenv
    antenv.axon_hooks = mod

    # keep profile artifacts local (no bucket access in this container)
    from concourse import bass_utils as _bu
    _bu.upload_artifacts = lambda tmpdir: tmpdir


def run(inputs, trace=False, **trace_kwargs):
    global LAST_EXEC_NS, LAST_RESULTS
    if trace:
        _ensure_profile_hook()
    nc = _get_nc()
    in_maps = shard_inputs(**inputs)
    res = run_bass_kernel_spmd(
        nc, in_maps, core_ids=list(range(N_CORES)), trace=trace, **trace_kwargs
    )
    LAST_EXEC_NS = res.exec_time_ns
    LAST_RESULTS = res
    ctx, aw = gather_outputs(res.results)
    return ctx, aw


def kernel(**inputs):
    trace = bool(int(os.environ.get("BAHDANAU_TRACE", "0")))
    ctx, aw = run(inputs, trace=trace)
    return ctx, aw


# revision 43
# speedup vs baseline: 1.0056x; 1.0056x over previous
"""Trainium2 Bass kernel: Bahdanau attention, data-parallel over batch on 8 NeuronCores.

kernel(**inputs) takes the full unsharded inputs (as in reference.setup_inputs())
and returns (context_vector [64, 2048] f32, attention_weights [64, 1024, 1] f32).

Sharding: batch 64 -> 8 per core; small weights replicated. Host-side work is
layout-only (slicing + transposes); all math runs on device.

Per-core device program (B_LOC = 8 batch items), all fp32:
  projhT[a,b] = b1[a] + b2[a] + sum_d W2[d,a] hidden[b,d]      (TensorE)
  ahT[a,l]    = tanh(sum_e W1[e,a] featT[e,l] + projhT[a,b])   (TensorE + ScalarE bias)
  score[l]    = sum_a V[a] ahT[a,l]                            (TensorE)
  E[l], S     = exp(score[l] + bv), sum_l E[l]                 (ScalarE accum_out)
  aw[l]       = E[l] * (1/S)                                   (VectorE)
  ctxT[e]     = (sum_l featT[e,l] E[l]) * (1/S)                (VectorE mult + ScalarE accum)

featT arrives pre-transposed [E, L] so the big matmul streams feature columns
with W1 tiles stationary; the context reduction runs on VectorE (elementwise
multiply) + ScalarE (Copy with accum_out row-reduce), overlapping the next
batch's TensorE work.

Hardware quirks honored here:
  - custom DVE microcode ops (tensor_tensor_reduce etc.) hang this target's
    exec units -> only standard DVE/ACT/PE instructions are used.
  - fp32 matmuls lower to LOW+HIGH double passes at half stream rate (4x
    slower than bf16, measured 858ns vs 213ns per [128]x[128,512] MM)
    -> the streaming matmuls (proj/score/Ebcast) run in bf16 with fp32 PSUM
    accumulation; biases, tanh/exp, softmax and outputs stay fp32.
  - an fp32 matmul is a single self-loading instruction with ONE sync-wait
    slot; tiny "absorber" matmuls/copies first-touch freshly DMA'd tiles so
    no real matmul ever needs two semaphore waits.
"""

import os
import numpy as np
import ml_dtypes
from contextlib import ExitStack

import concourse.bass as bass
import concourse.tile as tile
from concourse import bacc, mybir
from concourse.bass_utils import run_bass_kernel_spmd

FP32 = mybir.dt.float32
BF16 = mybir.dt.bfloat16
AF = mybir.ActivationFunctionType
ALU = mybir.AluOpType

N_CORES = 8
B_LOC, L, ENC, DEC, ATT = 8, 1024, 2048, 512, 512
NET = ENC // 128   # 16 e-tiles
NAT = ATT // 128   # 4 a-tiles
NDT = DEC // 128   # 4 d-tiles
NQ = 4             # feature "quarters" (4 e-tiles each) for DMA pipelining

LAST_EXEC_NS = None
LAST_RESULTS = None


def build_nc(debug=False):
    nc = bacc.Bacc(None, target_bir_lowering=False, debug=debug)

    featT = nc.declare_dram_parameter("featT", [B_LOC, ENC, L], BF16, isOutput=False)
    hiddenT = nc.declare_dram_parameter("hiddenT", [DEC, B_LOC], FP32, isOutput=False)
    W1 = nc.declare_dram_parameter("W1", [ENC, ATT], BF16, isOutput=False)
    W2 = nc.declare_dram_parameter("W2", [DEC, ATT], FP32, isOutput=False)
    b1 = nc.declare_dram_parameter("b1", [1, ATT], FP32, isOutput=False)
    b2 = nc.declare_dram_parameter("b2", [1, ATT], FP32, isOutput=False)
    V = nc.declare_dram_parameter("V", [ATT, 1], BF16, isOutput=False)
    bv = nc.declare_dram_parameter("bv", [1, 1], FP32, isOutput=False)
    ctx_out = nc.declare_dram_parameter("ctx_out", [128, B_LOC, NET], FP32, isOutput=True)
    aw_out = nc.declare_dram_parameter("aw_out", [B_LOC, L], FP32, isOutput=True)
    # internal DRAM bounce rows for the E-broadcast (partition-0-step APs are
    # only legal on DRAM); two slots so consecutive batches don't serialize
    ebounce = nc.dram_tensor("ebounce", [2, L], BF16)

    with ExitStack() as ctx:
        tc = ctx.enter_context(tile.TileContext(nc))
        singles = ctx.enter_context(tc.tile_pool(name="singles", bufs=1))
        ps_proj = ctx.enter_context(tc.tile_pool(name="ps_proj", bufs=3, space="PSUM"))
        ps_sc = ctx.enter_context(tc.tile_pool(name="ps_sc", bufs=1, space="PSUM"))

        def pe_absorb(tile_ap):
            """Tiny matmul whose only dependency is `tile_ap`'s producer --
            soaks up that wait on TensorE so the next real matmul needs at
            most one sync wait (fp32 MM hardware limit)."""
            dmy = ps_sc.tile([1, 2, 512], FP32, tag="sc")
            nc.tensor.matmul(
                dmy[0:1, 0, 0:1], tile_ap[0:1, 0:1], tile_ap[0:1, 0:1],
                start=True, stop=True,
            )

        def dve_absorb(tile_ap, junk):
            """Cheap copy that lands `tile_ap`'s DMA wait on VectorE early."""
            nc.vector.tensor_copy(junk[0:1, 0:1], tile_ap[0:1, 0:1])

        # ---------- preload persistent weights / constants ----------
        # streaming-matmul operands arrive as bf16 from the host; W1 is
        # loaded in four a-tile chunks so the first proj matmul only waits
        # for the chunk it needs
        W1sb = singles.tile([128, NAT, NET, 128], BF16)
        W1r = W1.rearrange("(t p) a -> p t a", p=128)
        for at in range(NAT):
            nc.sync.dma_start(
                out=W1sb[:, at, :, :], in_=W1r[:, :, at * 128:(at + 1) * 128]
            )
        Vsb = singles.tile([128, NAT], BF16)
        nc.sync.dma_start(out=Vsb[:], in_=V.rearrange("(t p) o -> p (t o)", p=128))
        bvsb = singles.tile([1, 1], FP32)
        nc.sync.dma_start(out=bvsb[:], in_=bv[:, :])
        ones128f = singles.tile([1, 128], FP32)
        nc.vector.memset(ones128f[:], 1.0)

        projhT = singles.tile([128, NAT, B_LOC], FP32)
        S_all = singles.tile([1, B_LOC], FP32)
        rS_all = singles.tile([1, B_LOC], FP32)
        ctxT = singles.tile([128, B_LOC, NET], FP32)
        junk = singles.tile([1, 1], FP32)

        # soak up the weight-load DMA waits on TensorE before any real matmul
        for at in range(NAT):
            pe_absorb(W1sb[0:1, at, 0, 0:1])
        pe_absorb(Vsb[0:1, 0:1])

        # ---------- projhT[a, b] = b1[a]+b2[a] + sum_d W2[d,a] hidden[b,d] ----------
        W2sb = singles.tile([128, NDT, ATT], FP32)
        nc.sync.dma_start(out=W2sb[:], in_=W2.rearrange("(t p) a -> p t a", p=128))
        hT = singles.tile([128, NDT, B_LOC], FP32)
        nc.sync.dma_start(out=hT[:], in_=hiddenT.rearrange("(t p) b -> p t b", p=128))
        bb_ = singles.tile([1, 2, ATT], FP32)
        nc.sync.dma_start(out=bb_[:, 0, :], in_=b1[:, :])
        nc.sync.dma_start(out=bb_[:, 1, :], in_=b2[:, :])
        ones8 = singles.tile([1, B_LOC], FP32)
        nc.vector.memset(ones8[:], 1.0)
        b12 = singles.tile([1, ATT], FP32)
        nc.vector.tensor_add(b12[:, :], bb_[:, 0, :], bb_[:, 1, :])

        pe_absorb(W2sb[0:1, 0, 0:1])
        pe_absorb(hT[0:1, 0, 0:1])

        for at in range(NAT):
            pph = ps_sc.tile([128, B_LOC], FP32, tag="sc")
            nc.tensor.matmul(
                pph[:, :],
                b12[0:1, at * 128:(at + 1) * 128],
                ones8[:, :],
                start=True, stop=False,
            )
            for dt_ in range(NDT):
                nc.tensor.matmul(
                    pph[:, :],
                    W2sb[:, dt_, at * 128:(at + 1) * 128],
                    hT[:, dt_, :],
                    start=False, stop=(dt_ == NDT - 1),
                )
            nc.scalar.activation(projhT[:, at, :], pph[:, :], AF.Copy)

        featp = ctx.enter_context(tc.tile_pool(name="featp", bufs=2 * NQ))
        ahp = ctx.enter_context(tc.tile_pool(name="ahp", bufs=2))
        ep = ctx.enter_context(tc.tile_pool(name="ep", bufs=1))
        awp = ctx.enter_context(tc.tile_pool(name="awp", bufs=2))
        scrp = ctx.enter_context(tc.tile_pool(name="scrp", bufs=3))
        ebp = ctx.enter_context(tc.tile_pool(name="ebp", bufs=2))

        # ---------- per-batch feature quarter loads ----------
        featq = [[None] * NQ for _ in range(B_LOC)]

        def load_quarter(b, q):
            t_ = featp.tile([128, NET // NQ, L], BF16, tag="fq")
            src = featT[b, q * 512:(q + 1) * 512, :].rearrange(
                "(t p) l -> p t l", p=128
            )
            nc.sync.dma_start(out=t_[:], in_=src)
            return t_

        for q in range(NQ):
            featq[0][q] = load_quarter(0, q)
        for q in range(NQ):
            pe_absorb(featq[0][q][0:1, 0, 0:1])
            dve_absorb(featq[0][q][0:1, 0, 0:1], junk)

        def emit_proj_at(b, ah, at):
            """One a-tile of proj + fused tanh into ahT."""
            pp = ps_proj.tile([128, 2, 512], FP32, tag="pp")
            for et in range(NET):
                q, t_ = et // NQ, et % NQ
                lhsT = W1sb[:, at, et, :]
                for x in range(2):
                    nc.tensor.matmul(
                        pp[:, x, :],
                        lhsT,
                        featq[b][q][:, t_, x * 512:(x + 1) * 512],
                        start=(et == 0), stop=(et == NET - 1),
                    )
            nc.scalar.activation(
                ah[:, at, :, :], pp[:, :, :], AF.Tanh,
                bias=projhT[:, at, b:b + 1],
            )

        def emit_score_a(b, ah):
            """score matmuls -> exp/S -> aw out -> bf16 row copy."""
            ps = ps_sc.tile([1, 2, 512], FP32, tag="sc")
            for at in range(NAT):
                for x in range(2):
                    nc.tensor.matmul(
                        ps[0:1, x, :],
                        Vsb[:, at:at + 1],
                        ah[:, at, x, :],
                        start=(at == 0), stop=(at == NAT - 1),
                    )

            # E = exp(score + bv), S = sum(E)  (softmax without max-subtraction:
            # |score| <= sum|V| + |bv| < 23, exp stays well inside fp32 range)
            E = ep.tile([1, 2, 512], FP32, tag="E")
            nc.scalar.activation(
                E[:, :, :], ps[0:1, :, :], AF.Exp,
                bias=bvsb[0:1, 0:1], accum_out=S_all[0:1, b:b + 1],
            )

            # aw = E / S  (output attention weights)
            nc.vector.reciprocal(rS_all[0:1, b:b + 1], S_all[0:1, b:b + 1])
            aw = awp.tile([1, 2, 512], FP32, tag="aw")
            nc.vector.tensor_scalar_mul(aw[:, :, :], E[:, :, :], rS_all[0:1, b:b + 1])
            nc.sync.dma_start(
                out=aw_out[b:b + 1, :].rearrange("o (x l) -> o x l", x=2),
                in_=aw[:],
            )
            E_bf = ep.tile([1, 2, 512], BF16, tag="Ebf")
            nc.vector.tensor_copy(E_bf[:, :, :], E[:, :, :])
            return E_bf

        def emit_score_b(E_bf):
            """E broadcast to all 128 partitions via SBUF->SBUF DMA with a
            zero-step partition access pattern (DMA engines are idle here;
            keeps TensorE/VectorE out of the broadcast entirely)."""
            from concourse.tile import add_dep_helper
            slot = emit_score_b.flip = getattr(emit_score_b, "flip", 0) ^ 1
            brow = ebounce[slot:slot + 1, :].rearrange("o (x l) -> o x l", x=2)
            d1 = nc.sync.dma_start(out=brow, in_=E_bf[:, :, :])
            eb_sb = ebp.tile([128, L], BF16, tag="ebsb")
            bsrc = bass.AP(
                tensor=brow.tensor, offset=brow.offset,
                ap=[[0, 128], [1, L]],
            )
            d2 = nc.sync.dma_start(out=eb_sb[:], in_=bsrc)
            # Tile does not track DRAM-tensor deps: order read-after-write
            add_dep_helper(d2.ins, d1.ins, True, "ebounce RAW")
            dve_absorb(eb_sb[0:1, 0:1], junk)
            return eb_sb

        def emit_ctx_chunk(b, eb_sb, ets, dve_reduce=True):
            """ctxT[:, b, et] = sum_l featT[e, l] * E[l] for et in ets:
            VectorE multiply into scratch; row-reduce alternates between
            ScalarE (Copy accum_out) and VectorE (tensor_reduce) unless
            dve_reduce is False (tail: keep VectorE free for multiplies)."""
            for et in ets:
                q, t_ = et // NQ, et % NQ
                f = featq[b][q]
                scr = scrp.tile([128, L], BF16, tag="scr")
                nc.vector.tensor_mul(scr[:, :], f[:, t_, :], eb_sb[:, :])
                if dve_reduce and et % 2 == 1:
                    nc.vector.tensor_reduce(
                        ctxT[:, b, et:et + 1], scr[:, :],
                        axis=mybir.AxisListType.X, op=ALU.add,
                    )
                else:
                    nc.scalar.activation(
                        scr[:, :], scr[:, :], AF.Copy,
                        accum_out=ctxT[:, b, et:et + 1],
                    )

        # ---------- main loop: batch b's proj interleaved with batch b-1's
        # score/softmax/context stages so TensorE never waits on the
        # exp->broadcast chain and HAM stays warm.
        # Per-iteration emission order:
        #   score(b-1) | proj(b).at0 | Ebcast(b-1) | proj(b).at1..3 with ctx
        #   chunks of b-1 interleaved | next-batch absorbers ----------
        pending = None  # (b-1, its ah tile)
        for b in range(B_LOC):
            if b + 1 < B_LOC:
                for q in range(NQ):
                    featq[b + 1][q] = load_quarter(b + 1, q)

            if pending is not None:
                pb, pah = pending
                E_bf = emit_score_a(pb, pah)
            ah = ahp.tile([128, NAT, 2, 512], BF16, tag="ah")
            emit_proj_at(b, ah, 0)
            if pending is not None:
                eb_sb = emit_score_b(E_bf)
            emit_proj_at(b, ah, 1)
            if pending is not None:
                emit_ctx_chunk(pb, eb_sb, range(0, 6))
            emit_proj_at(b, ah, 2)
            if pending is not None:
                emit_ctx_chunk(pb, eb_sb, range(6, 12))
            emit_proj_at(b, ah, 3)
            if pending is not None:
                emit_ctx_chunk(pb, eb_sb, range(12, NET))
            # first-touch absorbers for the next batch's feature quarters
            if b + 1 < B_LOC:
                for q in range(NQ):
                    pe_absorb(featq[b + 1][q][0:1, 0, 0:1])
                    dve_absorb(featq[b + 1][q][0:1, 0, 0:1], junk)
            pending = (b, ah)

        pb, pah = pending
        E_bf = emit_score_a(pb, pah)
        eb_sb = emit_score_b(E_bf)
        emit_ctx_chunk(pb, eb_sb, range(NET))

        # ---------- normalize ctx by 1/S and store ----------
        prsb = ps_sc.tile([128, B_LOC], FP32, tag="sc")
        nc.tensor.matmul(prsb[:, :], ones128f[:, :], rS_all[0:1, :], start=True, stop=True)
        rsb = singles.tile([128, B_LOC], FP32)
        nc.scalar.activation(rsb[:, :], prsb[:, :], AF.Copy)
        for b in range(B_LOC):
            nc.vector.tensor_scalar_mul(ctxT[:, b, :], ctxT[:, b, :], rsb[:, b:b + 1])
        nc.sync.dma_start(out=ctx_out[:, :, :], in_=ctxT[:, :, :])

    nc.compile()
    return nc


def shard_inputs(features, hidden_state, W1, b1, W2, b2, V, bv, n_cores=N_CORES):
    """Full inputs -> list of per-core in_maps (host-side layout/precision
    prep only: batch sharding, [L,E]->[E,L] transpose, bf16 cast of the
    streaming-matmul operands)."""
    features = np.ascontiguousarray(features, dtype=np.float32)
    B = features.shape[0]
    per = B // n_cores
    assert per == B_LOC
    bf = ml_dtypes.bfloat16
    w1 = np.ascontiguousarray(np.asarray(W1, np.float32).astype(bf))
    w2 = np.ascontiguousarray(W2, np.float32)
    b1r = np.ascontiguousarray(b1, np.float32).reshape(1, ATT)
    b2r = np.ascontiguousarray(b2, np.float32).reshape(1, ATT)
    vr = np.ascontiguousarray(np.asarray(V, np.float32).astype(bf)).reshape(ATT, 1)
    bvr = np.ascontiguousarray(bv, np.float32).reshape(1, 1)
    in_maps = []
    for c in range(n_cores):
        fs = features[c * per:(c + 1) * per]
        in_maps.append({
            "featT": np.ascontiguousarray(fs.transpose(0, 2, 1).astype(bf)),
            "hiddenT": np.ascontiguousarray(
                np.asarray(hidden_state[c * per:(c + 1) * per], np.float32).T
            ),
            "W1": w1, "W2": w2, "b1": b1r, "b2": b2r, "V": vr, "bv": bvr,
        })
    return in_maps


def gather_outputs(results):
    """Per-core result dicts -> (context_vector [B, ENC], attention_weights [B, L, 1])."""
    ctxs, aws = [], []
    for o in results:
        ctxs.append(
            np.ascontiguousarray(o["ctx_out"]).reshape(128, B_LOC, NET)
            .transpose(1, 2, 0).reshape(B_LOC, ENC)
        )
        aws.append(np.ascontiguousarray(o["aw_out"]).reshape(B_LOC, L, 1))
    return np.concatenate(ctxs, 0), np.concatenate(aws, 0)


_NC_CACHE = {}


def _get_nc():
    if "nc" not in _NC_CACHE:
        _NC_CACHE["nc"] = build_nc(debug=False)
    return _NC_CACHE["nc"]


def _ensure_profile_hook():
    """Provide antenv.axon_hooks + a ctypes NTFF profile hook when the
    environment ships a trimmed antenv (degrades silently when absent)."""
    import sys, types, contextlib, ctypes

    try:
        from antenv.axon_hooks import get_axon_ntff_profile_hook  # noqa: F401
        return
    except ImportError:
        pass

    so_path = "/opt/axon/libaxon_pjrt.so"
    if not os.path.exists(so_path):
        return
    lib = ctypes.CDLL(so_path)
    if not hasattr(lib, "axon_start_nrt_profile"):
        return
    lib.axon_start_nrt_profile.argtypes = [
        ctypes.POINTER(ctypes.c_int64), ctypes.c_size_t,
    ]
    lib.axon_start_nrt_profile.restype = ctypes.c_int64
    lib.axon_stop_nrt_profile.argtypes = [ctypes.c_char_p]
    lib.axon_stop_nrt_profile.restype = ctypes.c_int64

    @contextlib.contextmanager
    def _hook(output_dir, device_ids):
        import jax
        jax.devices()
        if device_ids:
            ids = (ctypes.c_int64 * len(device_ids))(*device_ids)
            rc = lib.axon_start_nrt_profile(ids, len(device_ids))
        else:
            rc = lib.axon_start_nrt_profile(None, 0)
        if rc != 0:
            raise RuntimeError(f"axon_start_nrt_profile rc={rc}")
        try:
            yield
        finally:
            n = lib.axon_stop_nrt_profile(str(output_dir).encode())
            if n < 0:
                raise RuntimeError(f"axon_stop_nrt_profile rc={n}")
            print(f"profile: {n} file(s) written to {output_dir}")

    mod = types.ModuleType("antenv.axon_hooks")
    _state = {"hook": _hook}
    mod.set_axon_ntff_profile_hook = lambda h: _state.__setitem__("hook", h)
    mod.get_axon_ntff_profile_hook = lambda: _state["hook"]
    sys.modules["antenv.axon_hooks"] = mod
    import antenv
    antenv.axon_hooks = mod

    # keep profile artifacts local (no bucket access in this container)
    from concourse import bass_utils as _bu
    _bu.upload_artifacts = lambda tmpdir: tmpdir


def run(inputs, trace=False, **trace_kwargs):
    global LAST_EXEC_NS, LAST_RESULTS
    if trace:
        _ensure_profile_hook()
    nc = _get_nc()
    in_maps = shard_inputs(**inputs)
    res = run_bass_kernel_spmd(
        nc, in_maps, core_ids=list(range(N_CORES)), trace=trace, **trace_kwargs
    )
    LAST_EXEC_NS = res.exec_time_ns
    LAST_RESULTS = res
    ctx, aw = gather_outputs(res.results)
    return ctx, aw


def kernel(**inputs):
    trace = bool(int(os.environ.get("BAHDANAU_TRACE", "0")))
    ctx, aw = run(inputs, trace=trace)
    return ctx, aw


# revision 44
# speedup vs baseline: 1.1164x; 1.1101x over previous
"""Trainium2 Bass kernel: Bahdanau attention, data-parallel over batch on 8 NeuronCores.

kernel(**inputs) takes the full unsharded inputs (as in reference.setup_inputs())
and returns (context_vector [64, 2048] f32, attention_weights [64, 1024, 1] f32).

Sharding: batch 64 -> 8 per core; small weights replicated. Host-side work is
layout-only (slicing + transposes); all math runs on device.

Per-core device program (B_LOC = 8 batch items), all fp32:
  projhT[a,b] = b1[a] + b2[a] + sum_d W2[d,a] hidden[b,d]      (TensorE)
  ahT[a,l]    = tanh(sum_e W1[e,a] featT[e,l] + projhT[a,b])   (TensorE + ScalarE bias)
  score[l]    = sum_a V[a] ahT[a,l]                            (TensorE)
  E[l], S     = exp(score[l] + bv), sum_l E[l]                 (ScalarE accum_out)
  aw[l]       = E[l] * (1/S)                                   (VectorE)
  ctxT[e]     = (sum_l featT[e,l] E[l]) * (1/S)                (VectorE mult + ScalarE accum)

featT arrives pre-transposed [E, L] so the big matmul streams feature columns
with W1 tiles stationary; the context reduction runs on VectorE (elementwise
multiply) + ScalarE (Copy with accum_out row-reduce), overlapping the next
batch's TensorE work.

Hardware quirks honored here:
  - custom DVE microcode ops (tensor_tensor_reduce etc.) hang this target's
    exec units -> only standard DVE/ACT/PE instructions are used.
  - fp32 matmuls lower to LOW+HIGH double passes at half stream rate (4x
    slower than bf16, measured 858ns vs 213ns per [128]x[128,512] MM)
    -> the streaming matmuls (proj/score/Ebcast) run in bf16 with fp32 PSUM
    accumulation; biases, tanh/exp, softmax and outputs stay fp32.
  - an fp32 matmul is a single self-loading instruction with ONE sync-wait
    slot; tiny "absorber" matmuls/copies first-touch freshly DMA'd tiles so
    no real matmul ever needs two semaphore waits.
"""

import os
import numpy as np
import ml_dtypes
from contextlib import ExitStack

import concourse.bass as bass
import concourse.tile as tile
from concourse import bacc, mybir
from concourse.bass_utils import run_bass_kernel_spmd

FP32 = mybir.dt.float32
BF16 = mybir.dt.bfloat16
AF = mybir.ActivationFunctionType
ALU = mybir.AluOpType

N_CORES = 8
B_LOC, L, ENC, DEC, ATT = 8, 1024, 2048, 512, 512
NET = ENC // 128   # 16 e-tiles
NAT = ATT // 128   # 4 a-tiles
NDT = DEC // 128   # 4 d-tiles
NQ = 4             # feature "quarters" (4 e-tiles each) for DMA pipelining

LAST_EXEC_NS = None
LAST_RESULTS = None


def build_nc(debug=False):
    nc = bacc.Bacc(None, target_bir_lowering=False, debug=debug)

    featT = nc.declare_dram_parameter("featT", [B_LOC, ENC, L], BF16, isOutput=False)
    hiddenT = nc.declare_dram_parameter("hiddenT", [DEC, B_LOC], FP32, isOutput=False)
    W1 = nc.declare_dram_parameter("W1", [ENC, ATT], BF16, isOutput=False)
    W2 = nc.declare_dram_parameter("W2", [DEC, ATT], FP32, isOutput=False)
    b1 = nc.declare_dram_parameter("b1", [1, ATT], FP32, isOutput=False)
    b2 = nc.declare_dram_parameter("b2", [1, ATT], FP32, isOutput=False)
    V = nc.declare_dram_parameter("V", [ATT, 1], BF16, isOutput=False)
    bv = nc.declare_dram_parameter("bv", [1, 1], FP32, isOutput=False)
    ctx_out = nc.declare_dram_parameter("ctx_out", [128, B_LOC, NET], FP32, isOutput=True)
    aw_out = nc.declare_dram_parameter("aw_out", [B_LOC, L], FP32, isOutput=True)
    # internal DRAM bounce rows for the E-broadcast (partition-0-step APs are
    # only legal on DRAM); two slots so consecutive batches don't serialize
    ebounce = nc.dram_tensor("ebounce", [2, L], BF16)

    with ExitStack() as ctx:
        tc = ctx.enter_context(tile.TileContext(nc))
        singles = ctx.enter_context(tc.tile_pool(name="singles", bufs=1))
        ps_proj = ctx.enter_context(tc.tile_pool(name="ps_proj", bufs=3, space="PSUM"))
        ps_sc = ctx.enter_context(tc.tile_pool(name="ps_sc", bufs=1, space="PSUM"))

        def pe_absorb(tile_ap):
            """Tiny matmul whose only dependency is `tile_ap`'s producer --
            soaks up that wait on TensorE so the next real matmul needs at
            most one sync wait (fp32 MM hardware limit)."""
            dmy = ps_sc.tile([1, 2, 512], FP32, tag="sc")
            nc.tensor.matmul(
                dmy[0:1, 0, 0:1], tile_ap[0:1, 0:1], tile_ap[0:1, 0:1],
                start=True, stop=True,
            )

        def dve_absorb(tile_ap, junk):
            """Cheap copy that lands `tile_ap`'s DMA wait on VectorE early."""
            nc.vector.tensor_copy(junk[0:1, 0:1], tile_ap[0:1, 0:1])

        # ---------- preload persistent weights / constants ----------
        # streaming-matmul operands arrive as bf16 from the host; W1 is
        # loaded in four a-tile chunks so the first proj matmul only waits
        # for the chunk it needs
        W1sb = singles.tile([128, NAT, NET, 128], BF16)
        W1r = W1.rearrange("(t p) a -> p t a", p=128)
        for at in range(NAT):
            nc.sync.dma_start(
                out=W1sb[:, at, :, :], in_=W1r[:, :, at * 128:(at + 1) * 128]
            )
        Vsb = singles.tile([128, NAT], BF16)
        nc.sync.dma_start(out=Vsb[:], in_=V.rearrange("(t p) o -> p (t o)", p=128))
        bvsb = singles.tile([1, 1], FP32)
        nc.sync.dma_start(out=bvsb[:], in_=bv[:, :])
        ones128f = singles.tile([1, 128], FP32)
        nc.vector.memset(ones128f[:], 1.0)

        projhT = singles.tile([128, NAT, B_LOC], FP32)
        S_all = singles.tile([1, B_LOC], FP32)
        rS_all = singles.tile([1, B_LOC], FP32)
        ctxT = singles.tile([128, B_LOC, NET], FP32)
        junk = singles.tile([1, 1], FP32)

        # soak up the weight-load DMA waits on TensorE before any real matmul
        for at in range(NAT):
            pe_absorb(W1sb[0:1, at, 0, 0:1])
        pe_absorb(Vsb[0:1, 0:1])

        # ---------- projhT[a, b] = b1[a]+b2[a] + sum_d W2[d,a] hidden[b,d] ----------
        W2sb = singles.tile([128, NDT, ATT], FP32)
        nc.sync.dma_start(out=W2sb[:], in_=W2.rearrange("(t p) a -> p t a", p=128))
        hT = singles.tile([128, NDT, B_LOC], FP32)
        nc.sync.dma_start(out=hT[:], in_=hiddenT.rearrange("(t p) b -> p t b", p=128))
        bb_ = singles.tile([1, 2, ATT], FP32)
        nc.sync.dma_start(out=bb_[:, 0, :], in_=b1[:, :])
        nc.sync.dma_start(out=bb_[:, 1, :], in_=b2[:, :])
        ones8 = singles.tile([1, B_LOC], FP32)
        nc.vector.memset(ones8[:], 1.0)
        b12 = singles.tile([1, ATT], FP32)
        nc.vector.tensor_add(b12[:, :], bb_[:, 0, :], bb_[:, 1, :])

        pe_absorb(W2sb[0:1, 0, 0:1])
        pe_absorb(hT[0:1, 0, 0:1])

        for at in range(NAT):
            pph = ps_sc.tile([128, B_LOC], FP32, tag="sc")
            nc.tensor.matmul(
                pph[:, :],
                b12[0:1, at * 128:(at + 1) * 128],
                ones8[:, :],
                start=True, stop=False,
            )
            for dt_ in range(NDT):
                nc.tensor.matmul(
                    pph[:, :],
                    W2sb[:, dt_, at * 128:(at + 1) * 128],
                    hT[:, dt_, :],
                    start=False, stop=(dt_ == NDT - 1),
                )
            nc.scalar.activation(projhT[:, at, :], pph[:, :], AF.Copy)

        featp = ctx.enter_context(tc.tile_pool(name="featp", bufs=2 * NQ + 2))
        ahp = ctx.enter_context(tc.tile_pool(name="ahp", bufs=3))
        ep = ctx.enter_context(tc.tile_pool(name="ep", bufs=1))
        awp = ctx.enter_context(tc.tile_pool(name="awp", bufs=2))
        scrp = ctx.enter_context(tc.tile_pool(name="scrp", bufs=3))
        ebp = ctx.enter_context(tc.tile_pool(name="ebp", bufs=2))

        # ---------- per-batch feature quarter loads ----------
        featq = [[None] * NQ for _ in range(B_LOC)]

        def load_quarter(b, q):
            t_ = featp.tile([128, NET // NQ, L], BF16, tag="fq")
            src = featT[b, q * 512:(q + 1) * 512, :].rearrange(
                "(t p) l -> p t l", p=128
            )
            nc.sync.dma_start(out=t_[:], in_=src)
            return t_

        for q in range(NQ):
            featq[0][q] = load_quarter(0, q)
        for q in range(NQ):
            pe_absorb(featq[0][q][0:1, 0, 0:1])
            dve_absorb(featq[0][q][0:1, 0, 0:1], junk)

        def emit_proj_at(b, ah, at):
            """One a-tile of proj + fused tanh into ahT."""
            pp = ps_proj.tile([128, 2, 512], FP32, tag="pp")
            for et in range(NET):
                q, t_ = et // NQ, et % NQ
                lhsT = W1sb[:, at, et, :]
                for x in range(2):
                    nc.tensor.matmul(
                        pp[:, x, :],
                        lhsT,
                        featq[b][q][:, t_, x * 512:(x + 1) * 512],
                        start=(et == 0), stop=(et == NET - 1),
                    )
            nc.scalar.activation(
                ah[:, at, :, :], pp[:, :, :], AF.Tanh,
                bias=projhT[:, at, b:b + 1],
            )

        def emit_score_a(b, ah):
            """score matmuls -> exp/S -> aw out -> bf16 row copy."""
            ps = ps_sc.tile([1, 2, 512], FP32, tag="sc")
            for at in range(NAT):
                for x in range(2):
                    nc.tensor.matmul(
                        ps[0:1, x, :],
                        Vsb[:, at:at + 1],
                        ah[:, at, x, :],
                        start=(at == 0), stop=(at == NAT - 1),
                    )

            # E = exp(score + bv), S = sum(E)  (softmax without max-subtraction:
            # |score| <= sum|V| + |bv| < 23, exp stays well inside fp32 range)
            E = ep.tile([1, 2, 512], FP32, tag="E")
            nc.scalar.activation(
                E[:, :, :], ps[0:1, :, :], AF.Exp,
                bias=bvsb[0:1, 0:1], accum_out=S_all[0:1, b:b + 1],
            )

            # aw = E / S  (output attention weights)
            nc.vector.reciprocal(rS_all[0:1, b:b + 1], S_all[0:1, b:b + 1])
            aw = awp.tile([1, 2, 512], FP32, tag="aw")
            nc.vector.tensor_scalar_mul(aw[:, :, :], E[:, :, :], rS_all[0:1, b:b + 1])
            nc.sync.dma_start(
                out=aw_out[b:b + 1, :].rearrange("o (x l) -> o x l", x=2),
                in_=aw[:],
            )
            E_bf = ep.tile([1, 2, 512], BF16, tag="Ebf")
            nc.vector.tensor_copy(E_bf[:, :, :], E[:, :, :])
            return E_bf

        def emit_score_b(E_bf):
            """E broadcast to all 128 partitions via SBUF->SBUF DMA with a
            zero-step partition access pattern (DMA engines are idle here;
            keeps TensorE/VectorE out of the broadcast entirely)."""
            from concourse.tile import add_dep_helper
            slot = emit_score_b.flip = getattr(emit_score_b, "flip", 0) ^ 1
            brow = ebounce[slot:slot + 1, :].rearrange("o (x l) -> o x l", x=2)
            d1 = nc.sync.dma_start(out=brow, in_=E_bf[:, :, :])
            eb_sb = ebp.tile([128, L], BF16, tag="ebsb")
            bsrc = bass.AP(
                tensor=brow.tensor, offset=brow.offset,
                ap=[[0, 128], [1, L]],
            )
            d2 = nc.sync.dma_start(out=eb_sb[:], in_=bsrc)
            # Tile does not track DRAM-tensor deps: order read-after-write
            add_dep_helper(d2.ins, d1.ins, True, "ebounce RAW")
            dve_absorb(eb_sb[0:1, 0:1], junk)
            return eb_sb

        def emit_ctx_chunk(b, eb_sb, ets, dve_reduce=True):
            """ctxT[:, b, et] = sum_l featT[e, l] * E[l] for et in ets:
            VectorE multiply into scratch; row-reduce alternates between
            ScalarE (Copy accum_out) and VectorE (tensor_reduce) unless
            dve_reduce is False (tail: keep VectorE free for multiplies)."""
            for et in ets:
                q, t_ = et // NQ, et % NQ
                f = featq[b][q]
                scr = scrp.tile([128, L], BF16, tag="scr")
                nc.vector.tensor_mul(scr[:, :], f[:, t_, :], eb_sb[:, :])
                if dve_reduce and et % 2 == 1:
                    nc.vector.tensor_reduce(
                        ctxT[:, b, et:et + 1], scr[:, :],
                        axis=mybir.AxisListType.X, op=ALU.add,
                    )
                else:
                    nc.scalar.activation(
                        scr[:, :], scr[:, :], AF.Copy,
                        accum_out=ctxT[:, b, et:et + 1],
                    )

        # ---------- main loop: batch b's proj interleaved with batch b-1's
        # score/softmax/context stages so TensorE never waits on the
        # exp->broadcast chain and HAM stays warm.
        # Per-iteration emission order:
        #   score(b-1) | proj(b).at0 | Ebcast(b-1) | proj(b).at1..3 with ctx
        #   chunks of b-1 interleaved | next-batch absorbers ----------
        pending = None  # (b-1, its ah tile)
        for b in range(B_LOC):
            if b + 1 < B_LOC:
                for q in range(NQ):
                    featq[b + 1][q] = load_quarter(b + 1, q)

            if pending is not None:
                pb, pah = pending
                E_bf = emit_score_a(pb, pah)
            ah = ahp.tile([128, NAT, 2, 512], BF16, tag="ah")
            emit_proj_at(b, ah, 0)
            if pending is not None:
                eb_sb = emit_score_b(E_bf)
            emit_proj_at(b, ah, 1)
            if pending is not None:
                emit_ctx_chunk(pb, eb_sb, range(0, 6))
            emit_proj_at(b, ah, 2)
            if pending is not None:
                emit_ctx_chunk(pb, eb_sb, range(6, 12))
            emit_proj_at(b, ah, 3)
            if pending is not None:
                emit_ctx_chunk(pb, eb_sb, range(12, NET))
            # first-touch absorbers for the next batch's feature quarters
            if b + 1 < B_LOC:
                for q in range(NQ):
                    pe_absorb(featq[b + 1][q][0:1, 0, 0:1])
                    dve_absorb(featq[b + 1][q][0:1, 0, 0:1], junk)
            pending = (b, ah)

        pb, pah = pending
        E_bf = emit_score_a(pb, pah)
        eb_sb = emit_score_b(E_bf)
        emit_ctx_chunk(pb, eb_sb, range(NET))

        # ---------- normalize ctx by 1/S and store ----------
        prsb = ps_sc.tile([128, B_LOC], FP32, tag="sc")
        nc.tensor.matmul(prsb[:, :], ones128f[:, :], rS_all[0:1, :], start=True, stop=True)
        rsb = singles.tile([128, B_LOC], FP32)
        nc.scalar.activation(rsb[:, :], prsb[:, :], AF.Copy)
        for b in range(B_LOC):
            nc.vector.tensor_scalar_mul(ctxT[:, b, :], ctxT[:, b, :], rsb[:, b:b + 1])
        nc.sync.dma_start(out=ctx_out[:, :, :], in_=ctxT[:, :, :])

    nc.compile()
    return nc


def shard_inputs(features, hidden_state, W1, b1, W2, b2, V, bv, n_cores=N_CORES):
    """Full inputs -> list of per-core in_maps (host-side layout/precision
    prep only: batch sharding, [L,E]->[E,L] transpose, bf16 cast of the
    streaming-matmul operands)."""
    features = np.ascontiguousarray(features, dtype=np.float32)
    B = features.shape[0]
    per = B // n_cores
    assert per == B_LOC
    bf = ml_dtypes.bfloat16
    w1 = np.ascontiguousarray(np.asarray(W1, np.float32).astype(bf))
    w2 = np.ascontiguousarray(W2, np.float32)
    b1r = np.ascontiguousarray(b1, np.float32).reshape(1, ATT)
    b2r = np.ascontiguousarray(b2, np.float32).reshape(1, ATT)
    vr = np.ascontiguousarray(np.asarray(V, np.float32).astype(bf)).reshape(ATT, 1)
    bvr = np.ascontiguousarray(bv, np.float32).reshape(1, 1)
    in_maps = []
    for c in range(n_cores):
        fs = features[c * per:(c + 1) * per]
        in_maps.append({
            "featT": np.ascontiguousarray(fs.transpose(0, 2, 1).astype(bf)),
            "hiddenT": np.ascontiguousarray(
                np.asarray(hidden_state[c * per:(c + 1) * per], np.float32).T
            ),
            "W1": w1, "W2": w2, "b1": b1r, "b2": b2r, "V": vr, "bv": bvr,
        })
    return in_maps


def gather_outputs(results):
    """Per-core result dicts -> (context_vector [B, ENC], attention_weights [B, L, 1])."""
    ctxs, aws = [], []
    for o in results:
        ctxs.append(
            np.ascontiguousarray(o["ctx_out"]).reshape(128, B_LOC, NET)
            .transpose(1, 2, 0).reshape(B_LOC, ENC)
        )
        aws.append(np.ascontiguousarray(o["aw_out"]).reshape(B_LOC, L, 1))
    return np.concatenate(ctxs, 0), np.concatenate(aws, 0)


_NC_CACHE = {}


def _get_nc():
    if "nc" not in _NC_CACHE:
        _NC_CACHE["nc"] = build_nc(debug=False)
    return _NC_CACHE["nc"]


def _ensure_profile_hook():
    """Provide antenv.axon_hooks + a ctypes NTFF profile hook when the
    environment ships a trimmed antenv (degrades silently when absent)."""
    import sys, types, contextlib, ctypes

    try:
        from antenv.axon_hooks import get_axon_ntff_profile_hook  # noqa: F401
        return
    except ImportError:
        pass

    so_path = "/opt/axon/libaxon_pjrt.so"
    if not os.path.exists(so_path):
        return
    lib = ctypes.CDLL(so_path)
    if not hasattr(lib, "axon_start_nrt_profile"):
        return
    lib.axon_start_nrt_profile.argtypes = [
        ctypes.POINTER(ctypes.c_int64), ctypes.c_size_t,
    ]
    lib.axon_start_nrt_profile.restype = ctypes.c_int64
    lib.axon_stop_nrt_profile.argtypes = [ctypes.c_char_p]
    lib.axon_stop_nrt_profile.restype = ctypes.c_int64

    @contextlib.contextmanager
    def _hook(output_dir, device_ids):
        import jax
        jax.devices()
        if device_ids:
            ids = (ctypes.c_int64 * len(device_ids))(*device_ids)
            rc = lib.axon_start_nrt_profile(ids, len(device_ids))
        else:
            rc = lib.axon_start_nrt_profile(None, 0)
        if rc != 0:
            raise RuntimeError(f"axon_start_nrt_profile rc={rc}")
        try:
            yield
        finally:
            n = lib.axon_stop_nrt_profile(str(output_dir).encode())
            if n < 0:
                raise RuntimeError(f"axon_stop_nrt_profile rc={n}")
            print(f"profile: {n} file(s) written to {output_dir}")

    mod = types.ModuleType("antenv.axon_hooks")
    _state = {"hook": _hook}
    mod.set_axon_ntff_profile_hook = lambda h: _state.__setitem__("hook", h)
    mod.get_axon_ntff_profile_hook = lambda: _state["hook"]
    sys.modules["antenv.axon_hooks"] = mod
    import antenv
    antenv.axon_hooks = mod

    # keep profile artifacts local (no bucket access in this container)
    from concourse import bass_utils as _bu
    _bu.upload_artifacts = lambda tmpdir: tmpdir


def run(inputs, trace=False, **trace_kwargs):
    global LAST_EXEC_NS, LAST_RESULTS
    if trace:
        _ensure_profile_hook()
    nc = _get_nc()
    in_maps = shard_inputs(**inputs)
    res = run_bass_kernel_spmd(
        nc, in_maps, core_ids=list(range(N_CORES)), trace=trace, **trace_kwargs
    )
    LAST_EXEC_NS = res.exec_time_ns
    LAST_RESULTS = res
    ctx, aw = gather_outputs(res.results)
    return ctx, aw


def kernel(**inputs):
    trace = bool(int(os.environ.get("BAHDANAU_TRACE", "0")))
    ctx, aw = run(inputs, trace=trace)
    return ctx, aw
